# revision 36
# baseline (speedup 1.0000x reference)
"""EvolveGCN-O forward pass on 8 Trainium2 NeuronCores (Bass/Tile).

Math (reference):
    w_new = LSTM-evolve(weight; w_ih, b_ih+b_hh)          # [C, C]
    out   = D^-1/2 (A + I) D^-1/2  X  w_new               # [N, C]

Key factorization: norm(j->i) = dinv[j]*dinv[i], so with xs = dinv*x
(prescaled on host, stored bf16):
    out[i] = dinv[i] * (sum_{j->i} xs[j] + xs[i]) @ w_new

Device strategy (edges + scatter targets sharded across 8 cores):
  * Destination nodes are greedily reassigned (LPT on per-range
    in-degree) to 128-node blocks so per-(block, range) edge counts are
    near-uniform (~2% padding); each core owns nbc blocks, processed in
    chunks of 7 (7 PSUM banks accumulate 7 blocks; the 8th is scratch).
    The host un-permutes the output rows at the end.
  * All gather metadata (int16 indices, bf16 dst labels) is DMAed into
    SBUF once up-front (~29 KB/partition), so the gather stream never
    waits on metadata loads.
  * Self-loop term: xs rows transposed into the block's PSUM
    accumulator via an identity matmul (start of each accumulation).
  * Edges: host sorts by dst block and splits by source range (the
    dma_gather index is a SIGNED int16 offset; 2 ranges cover N=100k).
    Per (block, range) cell padded to a uniform tile count; padding
    slots gather row 0 with dst-label -1 (masked in the selector).
  * Per edge tile of 128: gpsimd.dma_gather stages bf16 rows xs[src]
    (calls round-robin over 4 SWDGE queues); the vector engine builds
    ALL of a section's one-hot dst selectors in ONE batched is_equal
    (3-D broadcast against an iota tile, bf16); PE accumulates
    aggT += M^T @ S in fp32 PSUM from bf16 operands.
  * Per block: Y = (aggT^T @ w_new) scaled by dinv[dst] on the scalar
    engine during the PSUM->SBUF copy, then DMA out (fp32).
  * w_new computed on-device in fp32 (3 matmuls + activations, cast to
    bf16), redundantly per core. No collectives: block ownership makes
    outputs disjoint.
"""
import sys

for _p in ("/opt/trn_rl_repo", "/root/.axon_site/_ro/trn_rl_repo"):
    if _p not in sys.path:
        sys.path.append(_p)

import numpy as np

N, C, E = 100000, 128, 1600000  # problem shape (hardcoded per spec)
P = 128
N_CORES = 8
CHUNK = 7  # blocks per PSUM-resident chunk
IDX_WIN = 32768  # int16 signed reach below/above base
CALL_T = 8  # edge tiles per dma_gather call (1024 idx)
NQ = 4  # SWDGE queues (gather calls round-robin)


def _cdiv(a, b):
    return -(-a // b)


def _bf16(a):
    import ml_dtypes

    return np.ascontiguousarray(a.astype(ml_dtypes.bfloat16))


def _balance_blocks(d0, d1, npad, nb):
    """Greedy LPT assignment of nodes to nb blocks of 128 rows, balancing
    per-block sums of both per-range in-degrees.  Returns perm [npad]
    (slot -> node id) and pos [npad] (node id -> slot)."""
    import heapq

    n = len(d0)
    order = np.argsort(-(d0 + d1), kind="stable")
    heap = [(0, 0, 0, b, 0) for b in range(nb)]  # (maxload, c0, c1, id, cnt)
    heapq.heapify(heap)
    members = [[] for _ in range(nb)]
    for i in order:
        di0 = int(d0[i])
        di1 = int(d1[i])
        while True:
            mx, c0, c1, b, cnt = heapq.heappop(heap)
            if cnt < P:
                break  # full blocks are dropped permanently
        members[b].append(i)
        c0 += di0
        c1 += di1
        heapq.heappush(heap, (max(c0, c1), c0, c1, b, cnt + 1))
    pad_ids = iter(range(n, npad))
    perm = np.empty(npad, np.int64)
    for b in range(nb):
        m = members[b]
        while len(m) < P:
            m.append(next(pad_ids))
        perm[b * P : (b + 1) * P] = m
    pos = np.empty(npad, np.int64)
    pos[perm] = np.arange(npad)
    return perm, pos


def prep_inputs(x, edge_index, weight, w_ih, b_ih, b_hh, n=N):
    """Host-side sharding/index prep.

    Returns (in_maps, meta) where meta = (t_r tuple, nbc, chunk_sizes, npad).
    """
    x = np.asarray(x, dtype=np.float32)
    ei = np.asarray(edge_index)
    src_e = ei[0].astype(np.int64)
    dst_e = ei[1].astype(np.int64)

    # a couple of spare blocks per core so the balancer has headroom
    nbc = _cdiv(_cdiv(n, P), N_CORES) + 2
    nb = nbc * N_CORES
    npad = nb * P

    # degrees include self loops
    deg = (np.bincount(dst_e, minlength=n) + 1).astype(np.float32)
    dinv = (1.0 / np.sqrt(deg)).astype(np.float32)

    # prescaled node features, bf16 (natural node order: gather source)
    xs = x * dinv[:, None]
    xspad = np.zeros((npad, C), np.float32)
    xspad[:n] = xs
    xspad_bf = _bf16(xspad)

    # source ranges: split at the edge-count median so the two ranges get
    # equal traffic; both int16 windows must still cover their range.
    split = int(np.sort(src_e)[len(src_e) // 2])
    split = min(max(split, npad - 2 * IDX_WIN), 2 * IDX_WIN)
    bases = (max(0, split - IDX_WIN), max(0, npad - IDX_WIN))
    rng_of = (src_e >= split).astype(np.int64)

    # balanced dst-block assignment: equalize per-(block, range) edge
    # counts so the uniform tile caps carry almost no padding
    d0 = np.bincount(dst_e[rng_of == 0], minlength=n)
    d1deg = np.bincount(dst_e[rng_of == 1], minlength=n)
    perm, pos = _balance_blocks(d0, d1deg, npad, nb)

    d1 = np.zeros(npad, np.float32)
    d1[:n] = dinv
    d1 = d1[perm]  # slot order
    xself_perm = xspad_bf[perm]

    # sort edges by (block, range) then pack
    slot_e = pos[dst_e]
    blk = slot_e >> 7
    order = np.argsort(blk * 2 + rng_of, kind="stable")
    srcs = src_e[order]
    dsts_slot = slot_e[order]
    rngs = rng_of[order]

    # per-(block, range) counts -> uniform tile counts (+1 slack: the last
    # slot of every cell is then padding with idx 0, so no gather call can
    # end on a negative index, which the ucode would drop).
    cell = blk[order] * 2 + rngs
    counts = np.bincount(cell, minlength=nb * 2).reshape(nb, 2)
    t_r = tuple(int(_cdiv(int(counts[:, r].max()) + 1, P)) for r in range(2))
    cap = (t_r[0] * P, t_r[1] * P)

    chunk_sizes = [min(CHUNK, nbc - i) for i in range(0, nbc, CHUNK)]

    # pack edges into per-(block, range) padded slots
    cell_starts = np.zeros(nb * 2 + 1, np.int64)
    np.cumsum(counts.reshape(-1), out=cell_starts[1:])
    pos_in_cell = np.arange(len(srcs)) - cell_starts[cell]
    cell_base = np.zeros(nb * 2, np.int64)
    cell_base[0::2] = np.arange(nb) * (cap[0] + cap[1])
    cell_base[1::2] = cell_base[0::2] + cap[0]
    flat = cell_base[cell] + pos_in_cell

    tot = nb * (cap[0] + cap[1])
    idx_all = np.zeros(tot, np.int32)  # padding idx = 0 (valid row at base)
    dstl_all = np.full(tot, -1.0, np.float32)  # padding label -1 -> masked
    idx_all[flat] = (srcs - np.array(bases)[rngs]).astype(np.int32)
    dstl_all[flat] = (dsts_slot & (P - 1)).astype(np.float32)

    iota = np.broadcast_to(np.arange(P, dtype=np.float32), (P, P))
    ident = np.eye(P, dtype=np.float32)
    wt = np.asarray(weight, np.float32).T
    wiht = np.asarray(w_ih, np.float32).T
    bsum = (
        (np.asarray(b_ih, np.float32) + np.asarray(b_hh, np.float32))
        .reshape(4, C)
        .T.copy()
    )

    per_blk = cap[0] + cap[1]
    in_maps = []
    for m in range(N_CORES):
        lo_b = m * nbc
        seg = slice(lo_b * per_blk, (lo_b + nbc) * per_blk)
        idx_c = idx_all[seg].reshape(nbc, per_blk)
        dstl_c = dstl_all[seg].reshape(nbc, per_blk)

        # per-(chunk, range) sections, each a flat slot list
        gidx_secs = []
        gdstl_secs = []
        b0 = 0
        for cs in chunk_sizes:
            for r in range(2):
                off = 0 if r == 0 else cap[0]
                sec_idx = idx_c[b0 : b0 + cs, off : off + cap[r]].reshape(-1).copy()
                sec_dstl = (
                    dstl_c[b0 : b0 + cs, off : off + cap[r]].reshape(-1).copy()
                )
                # the gather ucode DROPS trailing negative indices per call;
                # real edges can be negative (range-relative).  Swap any
                # call-final negative idx with a padding slot (idx 0) of the
                # same (block,range) cell.
                sec_tiles = cs * t_r[r]
                call_ts = [CALL_T] * (sec_tiles // CALL_T)
                if sec_tiles % CALL_T:
                    call_ts.append(sec_tiles % CALL_T)
                ends = np.cumsum(np.array(call_ts)) * P - 1
                end_set = set(int(e) for e in ends)
                for s in ends:
                    s = int(s)
                    if sec_idx[s] >= 0:
                        continue
                    k = s // cap[r]
                    cnt = int(counts[lo_b + b0 + k, r])
                    for p in range(k * cap[r] + cnt, (k + 1) * cap[r]):
                        if p not in end_set:
                            for arr in (sec_idx, sec_dstl):
                                arr[s], arr[p] = arr[p], arr[s]
                            break
                    else:
                        raise RuntimeError("no swap slot for call-final pad")
                gidx_secs.append(sec_idx)
                gdstl_secs.append(sec_dstl)
            b0 += cs

        # idx wrap per CALL: i -> [i%16, i//16], replicated x8 across
        # partitions; concatenate calls/sections along free dim
        def wrap16(a):
            w = np.ascontiguousarray(a.reshape(-1, 16).T.astype(np.int16))
            return np.tile(w, (8, 1))  # [128, len/16]

        gidx_cols = []
        for sec in gidx_secs:
            st = len(sec) // P
            for c0 in range(0, st, CALL_T):
                ct = min(CALL_T, st - c0)
                gidx_cols.append(wrap16(sec[c0 * P : (c0 + ct) * P]))
        gidx = np.concatenate(gidx_cols, axis=1)

        # per-tile dst labels, partition-major per section: [128, tiles]
        gdstl = np.concatenate(
            [s.reshape(-1, P).T for s in gdstl_secs], axis=1
        )

        d1_c = d1[lo_b * P : (lo_b + nbc) * P].reshape(nbc, P).T.copy()

        in_maps.append(
            dict(
                xsrc=xspad_bf,
                xself=np.ascontiguousarray(
                    xself_perm[lo_b * P : (lo_b + nbc) * P]
                ),
                gidx=np.ascontiguousarray(gidx),
                gdstl=_bf16(gdstl),
                d1=np.ascontiguousarray(d1_c),
                iota=_bf16(iota),
                ident=_bf16(ident),
                wt=np.ascontiguousarray(wt),
                wiht=np.ascontiguousarray(wiht),
                bias=bsum,
            )
        )
    meta = (t_r, nbc, chunk_sizes, bases, npad, pos)
    return in_maps, meta


def build_program(meta, reps=1):
    import concourse.bacc as bacc
    import concourse.mybir as mybir
    import concourse.tile as tile

    t_r, nbc, chunk_sizes, bases, npad = meta[:5]
    f32 = mybir.dt.float32
    bf16 = mybir.dt.bfloat16
    i16 = mybir.dt.int16
    AF = mybir.ActivationFunctionType
    ALU = mybir.AluOpType

    n_tiles = nbc * (t_r[0] + t_r[1])  # edge tiles per core
    idx_w = n_tiles * P // 16  # gidx free dim
    max_sec_t = CHUNK * max(t_r)

    nc = bacc.Bacc("TRN2", num_swdge_queues=NQ)
    xsrc = nc.declare_dram_parameter("xsrc", [npad, C], bf16, isOutput=False)
    xself = nc.declare_dram_parameter("xself", [nbc * P, C], bf16, isOutput=False)
    gidx = nc.declare_dram_parameter("gidx", [P, idx_w], i16, isOutput=False)
    gdstl = nc.declare_dram_parameter("gdstl", [P, n_tiles], bf16, isOutput=False)
    d1 = nc.declare_dram_parameter("d1", [P, nbc], f32, isOutput=False)
    iota = nc.declare_dram_parameter("iota", [P, P], bf16, isOutput=False)
    ident = nc.declare_dram_parameter("ident", [P, P], bf16, isOutput=False)
    wt = nc.declare_dram_parameter("wt", [P, P], f32, isOutput=False)
    wiht = nc.declare_dram_parameter("wiht", [P, 4 * C], f32, isOutput=False)
    bias = nc.declare_dram_parameter("bias", [P, 4], f32, isOutput=False)
    out = nc.declare_dram_parameter("out", [nbc * P, C], f32, isOutput=True)

    with tile.TileContext(nc) as tc:
        with (
            tc.tile_pool(name="const", bufs=1) as constp,
            tc.tile_pool(name="stag", bufs=6) as stagp,
            tc.tile_pool(name="sel", bufs=3) as selp,
            tc.tile_pool(name="selfx", bufs=7) as selfp,
            tc.tile_pool(name="psA", bufs=CHUNK, space="PSUM") as psA,
            tc.tile_pool(name="psB", bufs=1, space="PSUM") as psB,
            tc.tile_pool(name="osb", bufs=6) as osbp,
        ):
            iota_sb = constp.tile([P, P], bf16, tag="iota")
            nc.sync.dma_start(out=iota_sb[:], in_=iota[:])
            ident_sb = constp.tile([P, P], bf16, tag="ident")
            nc.sync.dma_start(out=ident_sb[:], in_=ident[:])
            wt_sb = constp.tile([P, P], f32, tag="wt")
            nc.sync.dma_start(out=wt_sb[:], in_=wt[:])
            wiht_sb = constp.tile([P, 4 * C], f32, tag="wiht")
            nc.sync.dma_start(out=wiht_sb[:], in_=wiht[:])
            bias_sb = constp.tile([P, 4], f32, tag="bias")
            nc.sync.dma_start(out=bias_sb[:], in_=bias[:])
            d1_sb = constp.tile([P, nbc], f32, tag="d1")
            nc.sync.dma_start(out=d1_sb[:], in_=d1[:])
            # all gather metadata resident up-front: no per-section DMA waits
            gidx_sb = constp.tile([P, idx_w], i16, tag="gidx")
            nc.sync.dma_start(out=gidx_sb[:], in_=gidx[:])
            gdstl_sb = constp.tile([P, n_tiles], bf16, tag="gdstl")
            nc.sync.dma_start(out=gdstl_sb[:], in_=gdstl[:])

            # --- LSTM single step -> evolved weight w_new (bf16) ---
            gate_sb = {}
            for m, func, bcol in ((0, AF.Sigmoid, 0), (2, AF.Tanh, 2), (3, AF.Sigmoid, 3)):
                ps = psB.tile([P, P], f32, tag="psb")
                nc.tensor.matmul(
                    out=ps[:],
                    lhsT=wiht_sb[:, m * P : (m + 1) * P],
                    rhs=wt_sb[:],
                    start=True,
                    stop=True,
                )
                sb = constp.tile([P, P], f32, tag=f"gate{m}")
                nc.scalar.activation(
                    out=sb[:], in_=ps[:], func=func, bias=bias_sb[:, bcol : bcol + 1]
                )
                gate_sb[m] = sb
            cT = constp.tile([P, P], f32, tag="cT")
            nc.vector.tensor_mul(out=cT[:], in0=gate_sb[0][:], in1=gate_sb[2][:])
            tcT = constp.tile([P, P], f32, tag="tcT")
            nc.scalar.activation(out=tcT[:], in_=cT[:], func=AF.Tanh)
            wnT = constp.tile([P, P], bf16, tag="wnT")
            nc.vector.tensor_mul(out=wnT[:], in0=gate_sb[3][:], in1=tcT[:])
            wn_ps = psB.tile([P, P], f32, tag="psb")
            nc.tensor.matmul(
                out=wn_ps[:], lhsT=wnT[:], rhs=ident_sb[:], start=True, stop=True
            )
            wn_sb = constp.tile([P, P], bf16, tag="wn")
            nc.vector.tensor_copy(out=wn_sb[:], in_=wn_ps[:])

            # hoist num_idxs registers: one MOVE per distinct call size
            # instead of one per gather call
            call_sizes = {
                min(CALL_T, cs * t_r[r] - c0) * P
                for cs in chunk_sizes
                for r in range(2)
                for c0 in range(0, cs * t_r[r], CALL_T)
            }
            nidx_regs = {s: nc.gpsimd.to_reg(s) for s in sorted(call_sizes)}

            # --- main: chunks of blocks; self pass + 2 gather passes ---
            def emit_main(_iv=None):
                b0 = 0  # first block of chunk
                s_tile = 0  # global edge-tile cursor
                s_idx = 0  # global gidx column cursor (int16 cols)
                n_call = 0  # gather call counter (queue round-robin)
                for cs in chunk_sizes:
                    aggs = [
                        psA.tile([P, P], f32, name=f"agg{i}", tag="agg")
                        for i in range(cs)
                    ]
                    # self-loop pass: aggT[b] = xs_block^T
                    for i in range(cs):
                        b = b0 + i
                        xsb = selfp.tile([P, P], bf16, tag="xself")
                        nc.sync.dma_start(
                            out=xsb[:], in_=xself[b * P : (b + 1) * P, :]
                        )
                        nc.tensor.matmul(
                            out=aggs[i][:], lhsT=xsb[:], rhs=ident_sb[:],
                            start=True, stop=False,
                        )
                    # gather passes
                    for r in range(2):
                        sec_tiles = cs * t_r[r]
                        stag = stagp.tile([P, max_sec_t * P], bf16, tag="stag")
                        for c0 in range(0, sec_tiles, CALL_T):
                            ct = min(CALL_T, sec_tiles - c0)
                            nc.gpsimd.dma_gather(
                                out_ap=stag[:, c0 * P : (c0 + ct) * P].rearrange(
                                    "p (t c) -> p t c", t=ct
                                ),
                                in_ap=xsrc[bases[r] :, :],
                                idxs_ap=gidx_sb[
                                    :, s_idx + c0 * 8 : s_idx + (c0 + ct) * 8
                                ],
                                num_idxs=ct * P,
                                num_idxs_reg=nidx_regs[ct * P],
                                elem_size=P,
                                queue_num=n_call % NQ,
                            )
                            n_call += 1
                        # batched one-hot selectors for the whole section
                        sel = selp.tile([P, max_sec_t * P], bf16, tag="sel")
                        nc.vector.tensor_tensor(
                            out=sel[:, : sec_tiles * P].rearrange(
                                "p (t f) -> p t f", t=sec_tiles
                            ),
                            in0=gdstl_sb[:, s_tile : s_tile + sec_tiles]
                            .unsqueeze(2)
                            .broadcast_to([P, sec_tiles, P]),
                            in1=iota_sb[:]
                            .unsqueeze(1)
                            .broadcast_to([P, sec_tiles, P]),
                            op=ALU.is_equal,
                        )
                        # interleave across blocks so consecutive matmuls hit
                        # different PSUM banks (hides accumulation latency)
                        for t in range(t_r[r]):
                            for i in range(cs):
                                j = i * t_r[r] + t
                                nc.tensor.matmul(
                                    out=aggs[i][:],
                                    lhsT=stag[:, j * P : (j + 1) * P],
                                    rhs=sel[:, j * P : (j + 1) * P],
                                    start=False,
                                    stop=(r == 1 and t == t_r[r] - 1),
                                )
                        s_tile += sec_tiles
                        s_idx += sec_tiles * 8
                    # flush chunk
                    for i in range(cs):
                        b = b0 + i
                        agg_sb = osbp.tile([P, P], bf16, tag="aggsb")
                        nc.vector.tensor_copy(out=agg_sb[:], in_=aggs[i][:])
                        y_ps = psB.tile([P, P], f32, tag="psb")
                        nc.tensor.matmul(
                            out=y_ps[:], lhsT=agg_sb[:], rhs=wn_sb[:],
                            start=True, stop=True,
                        )
                        y_sb = osbp.tile([P, P], f32, tag="ysb")
                        nc.scalar.activation(
                            out=y_sb[:], in_=y_ps[:], func=AF.Copy,
                            scale=d1_sb[:, b : b + 1],
                        )
                        nc.sync.dma_start(
                            out=out[b * P : (b + 1) * P, :], in_=y_sb[:]
                        )
                    b0 += cs

            if reps > 1:
                with tc.For_i(0, reps, 1):
                    emit_main()
            else:
                emit_main()

    nc.finalize()
    return nc


def kernel(**inputs) -> np.ndarray:
    from concourse.bass_utils import run_bass_kernel_spmd

    x = inputs["x"]
    n = x.shape[0]
    in_maps, meta = prep_inputs(
        x,
        inputs["edge_index"],
        inputs["weight"],
        inputs["w_ih"],
        inputs["b_ih"],
        inputs["b_hh"],
        n=n,
    )
    nc = build_program(meta)
    res = run_bass_kernel_spmd(nc, in_maps, list(range(N_CORES)))
    full = np.concatenate([r["out"] for r in res.results], axis=0)
    pos = meta[5]
    return np.ascontiguousarray(full[pos[:n]])
